# revision 1
# baseline (speedup 1.0000x reference)
"""CWVAE Bass kernel for trn2: 3-level RSSM scan, replicated across 8 cores.

Reference semantics (per level, T steps, ctx_t = parent_det[t mod Tp]):
  h     = relu(concat(sample, ctx_t) @ Wp + bp)
  det'  = GRUCell(h, det)            (torch gate order r,z,n)
  qh    = relu(concat(det', obs_t) @ Wq + bq)
  sample= qh @ Wqm + bqm
  out_t = det'

v2 design notes:
  - fp16 matmul inputs (same PE rate as bf16, 3 more mantissa bits)
  - gi/gh emitted in two 512-det-column groups; each group's PSUM tile packs
    [r | z | in | hn] for those det columns so gate math starts after the
    first group's 36 matmuls instead of after all 72
  - gate chain runs in 256-col blocks: ACT sigmoid/tanh, DVE mul/add,
    GpSimd blend, then per-group PE transposes -> detT (fp16)
  - posterior accumulates obs k-tiles first (k=8..11), then det k-tiles, so
    PE has ready work while late det blocks finish
  - per-step PSUM tiles rotate through one 2-slot pool tag (8 banks total)
"""
import sys

sys.path.insert(0, "/opt/trn_rl_repo")

import numpy as np

import concourse.bass as bass
import concourse.tile as tile
from concourse import bacc, mybir
from concourse.bass import ds
from concourse.bass_utils import run_bass_kernel_spmd
from concourse.masks import make_identity

F32 = mybir.dt.float32
F16 = mybir.dt.float16
AF = mybir.ActivationFunctionType

B = 128
DET = 1024
EMB = 512
STO = 128
OBS = 512
G = 3072
TS = [256, 64, 16]
KD = DET // 128   # 8
KE = EMB // 128   # 4
KQ = (DET + OBS) // 128  # 12
W = 8             # unroll (steps per inner loop body)


def build_kernel(has_gate_bias=False, has_bqm=False, has_pq_bias=True):
    nc = bacc.Bacc()

    inp = {}
    for l in range(3):
        T = TS[l]
        inp[f"wps{l}"] = nc.dram_tensor(f"wps{l}", [128, EMB], F16, kind="ExternalInput")
        if l != 2:
            inp[f"wpc{l}"] = nc.dram_tensor(f"wpc{l}", [KD, 128, EMB], F16, kind="ExternalInput")
        inp[f"wih{l}"] = nc.dram_tensor(f"wih{l}", [KE, 128, G], F16, kind="ExternalInput")
        inp[f"whh{l}"] = nc.dram_tensor(f"whh{l}", [KD, 128, G], F16, kind="ExternalInput")
        inp[f"wq{l}"] = nc.dram_tensor(f"wq{l}", [KQ, 128, EMB], F16, kind="ExternalInput")
        inp[f"wqm{l}"] = nc.dram_tensor(f"wqm{l}", [KE, 128, STO], F16, kind="ExternalInput")
        inp[f"obst{l}"] = nc.dram_tensor(f"obst{l}", [T, KE, 128, B], F16, kind="ExternalInput")
        inp[f"bp{l}"] = nc.dram_tensor(f"bp{l}", [EMB], F32, kind="ExternalInput")
        inp[f"bq{l}"] = nc.dram_tensor(f"bq{l}", [EMB], F32, kind="ExternalInput")
        inp[f"bqm{l}"] = nc.dram_tensor(f"bqm{l}", [STO], F32, kind="ExternalInput")
        if has_gate_bias:
            # packed [brz(2048) | bin(1024) | bhn(1024)] in fp16
            inp[f"bg{l}"] = nc.dram_tensor(f"bg{l}", [4096], F16, kind="ExternalInput")

    y = nc.dram_tensor("y", [B, TS[0], DET], F32, kind="ExternalOutput")

    detT_store = {
        2: nc.dram_tensor("detT2", [TS[2], KD, 128, B], F16),
        1: nc.dram_tensor("detT1", [TS[1], KD, 128, B], F16),
    }
    hctx = {
        1: nc.dram_tensor("hctx1", [TS[2], KE, 128, B], F32),
        0: nc.dram_tensor("hctx0", [TS[1], KE, 128, B], F32),
    }

    from contextlib import ExitStack
    with tile.TileContext(nc) as tc, ExitStack() as stk:
        const = stk.enter_context(tc.tile_pool(name="const", bufs=1))
        wts = stk.enter_context(tc.tile_pool(name="wts", bufs=1))
        state = stk.enter_context(tc.tile_pool(name="state", bufs=1))
        sb = stk.enter_context(tc.tile_pool(name="sb", bufs=3))
        gsb = stk.enter_context(tc.tile_pool(name="gsb", bufs=2))
        ps = stk.enter_context(tc.tile_pool(name="ps", bufs=2, space="PSUM"))

        ident = const.tile([128, 128], F32)
        make_identity(nc, ident)
        if has_gate_bias:
            ones_row = const.tile([1, 128], F16)
            nc.vector.memset(ones_row, 1.0)

        det32 = state.tile([128, DET], F32)
        detT = state.tile([128, DET], F16)
        qmT = state.tile([128, STO], F16)
        hT = state.tile([128, EMB], F16)
        qhT = state.tile([128, EMB], F16)

        for l in (2, 1, 0):
            T = TS[l]
            w_ps = wts.tile([128, EMB], F16, tag="w_ps")
            nc.sync.dma_start(out=w_ps, in_=inp[f"wps{l}"][:, :])
            w_ih = wts.tile([128, KE * G], F16, tag="w_ih")
            nc.sync.dma_start(out=w_ih[:, :].rearrange("p (k g) -> p k g", k=KE), in_=inp[f"wih{l}"].rearrange("k p g -> p k g"))
            w_hh = wts.tile([128, KD * G], F16, tag="w_hh")
            nc.sync.dma_start(out=w_hh[:, :].rearrange("p (k g) -> p k g", k=KD), in_=inp[f"whh{l}"].rearrange("k p g -> p k g"))
            w_q = wts.tile([128, KQ * EMB], F16, tag="w_q")
            nc.sync.dma_start(out=w_q[:, :].rearrange("p (k e) -> p k e", k=KQ), in_=inp[f"wq{l}"].rearrange("k p e -> p k e"))
            w_qm = wts.tile([128, KE * STO], F16, tag="w_qm")
            nc.sync.dma_start(out=w_qm[:, :].rearrange("p (k s) -> p k s", k=KE), in_=inp[f"wqm{l}"].rearrange("k p s -> p k s"))
            b_p = wts.tile([128, KE], F32, tag="b_p")
            nc.sync.dma_start(out=b_p, in_=inp[f"bp{l}"].rearrange("(m p) -> p m", p=128))
            b_q = wts.tile([128, KE], F32, tag="b_q")
            nc.sync.dma_start(out=b_q, in_=inp[f"bq{l}"].rearrange("(m p) -> p m", p=128))
            b_qm = wts.tile([128, 1], F32, tag="b_qm")
            nc.sync.dma_start(out=b_qm, in_=inp[f"bqm{l}"].rearrange("(m p) -> p m", p=128))
            if has_gate_bias:
                b_g = wts.tile([1, 4096], F16, tag="b_g")
                nc.sync.dma_start(out=b_g, in_=inp[f"bg{l}"].rearrange("(o g) -> o g", o=1))

            # ---- precompute hctx[l] from parent detT store ----
            if l != 2:
                w_pc = wts.tile([128, KD * EMB], F16, tag="w_pc")
                nc.sync.dma_start(out=w_pc[:, :].rearrange("p (k e) -> p k e", k=KD), in_=inp[f"wpc{l}"].rearrange("k p e -> p k e"))
                Tp = TS[l + 1]
                src = detT_store[l + 1]
                dst = hctx[l]
                for sc in range(Tp // 4):
                    s4 = sc * 4
                    rhs = gsb.tile([128, KD * 512], F16, tag="pre_rhs")
                    for k in range(KD):
                        nc.sync.dma_start(
                            out=rhs[:, k * 512:(k + 1) * 512].rearrange("p (s b) -> p s b", s=4),
                            in_=src[s4:s4 + 4, k, :, :].rearrange("s p b -> p s b"),
                        )
                    pg = ps.tile([128, 2048], F32, tag="pg")
                    for m in range(KE):
                        for k in range(KD):
                            nc.tensor.matmul(
                                pg[:, m * 512:(m + 1) * 512],
                                w_pc[:, (k * KE + m) * 128:(k * KE + m) * 128 + 128],
                                rhs[:, k * 512:(k + 1) * 512],
                                start=(k == 0),
                                stop=(k == KD - 1),
                            )
                    hc = gsb.tile([128, 2048], F32, tag="pre_hc")
                    for m in range(KE):
                        nc.scalar.copy(out=hc[:, m * 512:(m + 1) * 512], in_=pg[:, m * 512:(m + 1) * 512])
                    for sl in range(4):
                        nc.sync.dma_start(
                            out=dst[s4 + sl, :, :, :].rearrange("m p b -> p m b"),
                            in_=hc[:, :].rearrange("p (m s b) -> p m s b", m=KE, s=4)[:, :, sl, :],
                        )

            # ---- scan ----
            nc.vector.memset(det32, 0.0)
            nc.vector.memset(detT, 0.0)
            nc.vector.memset(qmT, 0.0)
            has_ctx = l != 2
            is_out = l == 0

            def step_body(t, eidx):
                obst = sb.tile([128, OBS], F16, tag="obst")
                nc.sync.dma_start(
                    out=obst[:, :].rearrange("p (k b) -> p k b", k=KE),
                    in_=inp[f"obst{l}"][ds(t, 1), :, :, :].rearrange("o k p b -> (o p) k b"),
                )
                # --- prior (feature-major) ---
                ppr = ps.tile([128, EMB], F32, tag="pg")
                for m in range(KE):
                    nc.tensor.matmul(
                        ppr[:, m * 128:(m + 1) * 128],
                        w_ps[:, m * 128:(m + 1) * 128],
                        qmT, start=True, stop=True,
                    )
                if has_ctx:
                    hcs = sb.tile([128, EMB], F32, tag="hcs")
                    nc.sync.dma_start(
                        out=hcs[:, :].rearrange("p (m b) -> p m b", m=KE),
                        in_=hctx[l][ds(eidx, 1), :, :, :].rearrange("o m p b -> (o p) m b"),
                    )
                    tpr = sb.tile([128, EMB], F32, tag="tpr")
                    nc.vector.tensor_add(out=tpr, in0=ppr, in1=hcs)
                    prsrc = tpr
                else:
                    prsrc = ppr
                for m in range(KE):
                    nc.scalar.activation(
                        out=hT[:, m * 128:(m + 1) * 128],
                        in_=prsrc[:, m * 128:(m + 1) * 128],
                        func=AF.Relu, bias=b_p[:, m:m + 1],
                    )
                # --- gi/gh in two det-column groups ---
                # pg_g layout (per group g, det cols g*512:(g+1)*512):
                #   [0:512]=r  [512:1024]=z  [1024:1536]=in  [1536:2048]=hn
                pgs = []
                for g in range(2):
                    pg_g = ps.tile([128, 2048], F32, tag="pg")
                    pgs.append(pg_g)
                    banks = [
                        (0, g * 512, True, True),             # r
                        (512, 1024 + g * 512, True, True),    # z
                        (1024, 2048 + g * 512, True, False),  # in (gi only)
                        (1536, 2048 + g * 512, False, True),  # hn (gh only)
                    ]
                    for bi, (dst, col, use_gi, use_gh) in enumerate(banks):
                        out_ap = pg_g[:, dst:dst + 512]
                        first = True
                        if use_gi:
                            for k in range(KE):
                                nc.tensor.matmul(
                                    out_ap, hT[:, k * 128:(k + 1) * 128],
                                    w_ih[:, k * G + col: k * G + col + 512],
                                    start=first,
                                    stop=(not use_gh and k == KE - 1 and not has_gate_bias),
                                )
                                first = False
                        if use_gh:
                            for k in range(KD):
                                nc.tensor.matmul(
                                    out_ap, detT[:, k * 128:(k + 1) * 128],
                                    w_hh[:, k * G + col: k * G + col + 512],
                                    start=first,
                                    stop=(k == KD - 1 and not has_gate_bias),
                                )
                                first = False
                        if has_gate_bias:
                            if col < 2048:
                                bcol = col
                            elif use_gi:
                                bcol = 2048 + g * 512
                            else:
                                bcol = 3072 + g * 512
                            nc.tensor.matmul(out_ap, ones_row, b_g[:, bcol:bcol + 512],
                                             start=False, stop=True)
                # --- gates: 4 blocks of 256 det cols, pipelined ---
                r_sb = gsb.tile([128, 1024], F32, tag="r_sb")
                z_sb = gsb.tile([128, 1024], F32, tag="z_sb")
                n_sb = gsb.tile([128, 1024], F32, tag="n_sb")
                d_sb = gsb.tile([128, 1024], F32, tag="d_sb")
                for g in range(2):
                    pg_g = pgs[g]
                    for bb in range(2):
                        bl = g * 2 + bb
                        c = bl * 256            # det col offset of this block
                        o = bb * 256            # offset within group tile
                        rs = r_sb[:, c:c + 256]
                        zs = z_sb[:, c:c + 256]
                        ns = n_sb[:, c:c + 256]
                        dsl = d_sb[:, c:c + 256]
                        nc.scalar.activation(out=rs, in_=pg_g[:, o:o + 256], func=AF.Sigmoid)
                        nc.scalar.activation(out=zs, in_=pg_g[:, 512 + o:512 + o + 256], func=AF.Sigmoid)
                        nc.vector.tensor_mul(out=ns, in0=rs, in1=pg_g[:, 1536 + o:1536 + o + 256])
                        nc.vector.tensor_add(out=ns, in0=ns, in1=pg_g[:, 1024 + o:1024 + o + 256])
                        nc.scalar.activation(out=ns, in_=ns, func=AF.Tanh)
                        nc.gpsimd.tensor_sub(out=dsl, in0=det32[:, c:c + 256], in1=ns)
                        nc.gpsimd.tensor_mul(out=dsl, in0=zs, in1=dsl)
                        nc.gpsimd.tensor_add(out=det32[:, c:c + 256], in0=ns, in1=dsl)
                    # transposes for this group's four 128-col tiles
                    ptp = ps.tile([128, 512], F32, tag="pg")
                    for kk in range(4):
                        k = g * 4 + kk
                        nc.tensor.transpose(ptp[:, kk * 128:(kk + 1) * 128],
                                            det32[:, k * 128:(k + 1) * 128], ident)
                    for kk in range(4):
                        k = g * 4 + kk
                        nc.vector.tensor_copy(out=detT[:, k * 128:(k + 1) * 128],
                                              in_=ptp[:, kk * 128:(kk + 1) * 128])
                if is_out:
                    nc.sync.dma_start(
                        out=y[:, ds(t, 1), :].rearrange("p o d -> p (o d)"),
                        in_=det32,
                    )
                else:
                    nc.sync.dma_start(
                        out=detT_store[l][ds(t, 1), :, :, :].rearrange("o k p b -> (o p) k b"),
                        in_=detT[:, :].rearrange("p (k b) -> p k b", k=KD),
                    )
                # --- posterior (feature-major); obs k-tiles first ---
                korder = list(range(KD, KQ)) + list(range(KD))
                ppo = ps.tile([128, EMB], F32, tag="pg")
                for m in range(KE):
                    for i, k in enumerate(korder):
                        rhs = detT[:, k * 128:(k + 1) * 128] if k < KD else obst[:, (k - KD) * 128:(k - KD + 1) * 128]
                        nc.tensor.matmul(
                            ppo[:, m * 128:(m + 1) * 128],
                            w_q[:, (k * KE + m) * 128:(k * KE + m) * 128 + 128],
                            rhs, start=(i == 0), stop=(i == KQ - 1),
                        )
                for m in range(KE):
                    nc.scalar.activation(
                        out=qhT[:, m * 128:(m + 1) * 128],
                        in_=ppo[:, m * 128:(m + 1) * 128],
                        func=AF.Relu, bias=b_q[:, m:m + 1],
                    )
                # --- head ---
                phd = ps.tile([128, STO], F32, tag="pg")
                for k in range(KE):
                    nc.tensor.matmul(
                        phd, w_qm[:, k * 128:(k + 1) * 128],
                        qhT[:, k * 128:(k + 1) * 128],
                        start=(k == 0), stop=(k == KE - 1),
                    )
                if has_bqm:
                    nc.scalar.activation(out=qmT, in_=phd, func=AF.Identity, bias=b_qm[:, 0:1])
                else:
                    nc.scalar.copy(out=qmT, in_=phd)

            hints = (mybir.EngineType.PE,)
            if not has_ctx:
                with tc.For_i(0, T // W, 1, hint_engines=hints) as sp:
                    for j in range(W):
                        step_body(sp * W + j, None)
            else:
                Tp = TS[l + 1]
                for a in range(T // Tp):
                    with tc.For_i(0, Tp // W, 1, hint_engines=hints) as sp:
                        for j in range(W):
                            e = sp * W + j
                            step_body(a * Tp + e, e)

    nc.compile()
    return nc


def prep_inputs(inputs, has_gate_bias=False):
    f16 = np.float16
    m = {}
    obs = [inputs["obs_l0"], inputs["obs_l1"], inputs["obs_l2"]]
    for l in range(3):
        T = TS[l]
        Wp, Wih, Whh = inputs["Wp"][l], inputs["Wih"][l], inputs["Whh"][l]
        Wq, Wqm = inputs["Wq"][l], inputs["Wqm"][l]
        m[f"wps{l}"] = np.ascontiguousarray(Wp[0:128]).astype(f16)
        if l != 2:
            m[f"wpc{l}"] = np.ascontiguousarray(Wp[128:1152].reshape(KD, 128, EMB)).astype(f16)
        m[f"wih{l}"] = np.ascontiguousarray(Wih.reshape(KE, 128, G)).astype(f16)
        m[f"whh{l}"] = np.ascontiguousarray(Whh.reshape(KD, 128, G)).astype(f16)
        m[f"wq{l}"] = np.ascontiguousarray(Wq.reshape(KQ, 128, EMB)).astype(f16)
        m[f"wqm{l}"] = np.ascontiguousarray(Wqm.reshape(KE, 128, STO)).astype(f16)
        o = np.asarray(obs[l], np.float32)
        m[f"obst{l}"] = np.ascontiguousarray(o.transpose(1, 2, 0).reshape(T, KE, 128, B)).astype(f16)
        m[f"bp{l}"] = np.ascontiguousarray(inputs["bp"][l], dtype=np.float32)
        m[f"bq{l}"] = np.ascontiguousarray(inputs["bq"][l], dtype=np.float32)
        m[f"bqm{l}"] = np.ascontiguousarray(inputs["bqm"][l], dtype=np.float32)
        if has_gate_bias:
            bih, bhh = np.asarray(inputs["bih"][l], np.float32), np.asarray(inputs["bhh"][l], np.float32)
            bg = np.concatenate([(bih + bhh)[0:2048], bih[2048:3072], bhh[2048:3072]])
            m[f"bg{l}"] = bg.astype(f16)
    return m


_CACHE = {}


def kernel(**inputs):
    inputs = {k: np.asarray(v) for k, v in inputs.items()}
    key = flags_for(inputs)
    if key not in _CACHE:
        _CACHE[key] = build_kernel(*key)
    nc = _CACHE[key]
    in_map = prep_inputs(inputs, key[0])
    res = run_bass_kernel_spmd(nc, [in_map for _ in range(8)], core_ids=list(range(8)))
    return res.results[0]["y"].astype(np.float32)


def flags_for(inputs):
    import numpy as _np
    return (
        bool(_np.any(inputs["bih"]) or _np.any(inputs["bhh"])),
        bool(_np.any(inputs["bqm"])),
        bool(_np.any(inputs["bp"]) or _np.any(inputs["bq"])),
    )



# revision 5
# speedup vs baseline: 1.0154x; 1.0154x over previous
"""CWVAE Bass kernel for trn2: 3-level RSSM scan, replicated across 8 cores.

Reference semantics (per level, T steps, ctx_t = parent_det[t mod Tp]):
  h     = relu(concat(sample, ctx_t) @ Wp + bp)
  det'  = GRUCell(h, det)            (torch gate order r,z,n)
  qh    = relu(concat(det', obs_t) @ Wq + bq)
  sample= qh @ Wqm + bqm
  out_t = det'

v2 design notes:
  - fp16 matmul inputs (same PE rate as bf16, 3 more mantissa bits)
  - gi/gh emitted in two 512-det-column groups; each group's PSUM tile packs
    [r | z | in | hn] for those det columns so gate math starts after the
    first group's 36 matmuls instead of after all 72
  - gate chain runs in 256-col blocks: ACT sigmoid/tanh, DVE mul/add,
    GpSimd blend, then per-group PE transposes -> detT (fp16)
  - posterior accumulates obs k-tiles first (k=8..11), then det k-tiles, so
    PE has ready work while late det blocks finish
  - per-step PSUM tiles rotate through one 2-slot pool tag (8 banks total)
"""
import sys

sys.path.insert(0, "/opt/trn_rl_repo")

import numpy as np

import concourse.bass as bass
import concourse.tile as tile
from concourse import bacc, mybir
from concourse.bass import ds
from concourse.bass_utils import run_bass_kernel_spmd
from concourse.masks import make_identity

F32 = mybir.dt.float32
F16 = mybir.dt.float16
AF = mybir.ActivationFunctionType

B = 128
DET = 1024
EMB = 512
STO = 128
OBS = 512
G = 3072
TS = [256, 64, 16]
KD = DET // 128   # 8
KE = EMB // 128   # 4
KQ = (DET + OBS) // 128  # 12
W = 16            # unroll (steps per inner loop body)


def build_kernel(has_gate_bias=False, has_bqm=False, has_pq_bias=True):
    nc = bacc.Bacc()

    inp = {}
    for l in range(3):
        T = TS[l]
        inp[f"wps{l}"] = nc.dram_tensor(f"wps{l}", [128, EMB], F16, kind="ExternalInput")
        if l != 2:
            inp[f"wpc{l}"] = nc.dram_tensor(f"wpc{l}", [KD, 128, EMB], F16, kind="ExternalInput")
        inp[f"wih{l}"] = nc.dram_tensor(f"wih{l}", [KE, 128, G], F16, kind="ExternalInput")
        inp[f"whh{l}"] = nc.dram_tensor(f"whh{l}", [KD, 128, G], F16, kind="ExternalInput")
        inp[f"wq{l}"] = nc.dram_tensor(f"wq{l}", [KQ, 128, EMB], F16, kind="ExternalInput")
        inp[f"wqm{l}"] = nc.dram_tensor(f"wqm{l}", [KE, 128, STO], F16, kind="ExternalInput")
        inp[f"obst{l}"] = nc.dram_tensor(f"obst{l}", [T, KE, 128, B], F16, kind="ExternalInput")
        inp[f"bp{l}"] = nc.dram_tensor(f"bp{l}", [EMB], F32, kind="ExternalInput")
        inp[f"bq{l}"] = nc.dram_tensor(f"bq{l}", [EMB], F32, kind="ExternalInput")
        inp[f"bqm{l}"] = nc.dram_tensor(f"bqm{l}", [STO], F32, kind="ExternalInput")
        if has_gate_bias:
            # packed [brz(2048) | bin(1024) | bhn(1024)] in fp16
            inp[f"bg{l}"] = nc.dram_tensor(f"bg{l}", [4096], F16, kind="ExternalInput")

    y = nc.dram_tensor("y", [B, TS[0], DET], F32, kind="ExternalOutput")

    detT_store = {
        2: nc.dram_tensor("detT2", [TS[2], KD, 128, B], F16),
        1: nc.dram_tensor("detT1", [TS[1], KD, 128, B], F16),
    }
    hctx = {
        1: nc.dram_tensor("hctx1", [TS[2], KE, 128, B], F32),
        0: nc.dram_tensor("hctx0", [TS[1], KE, 128, B], F32),
    }

    from contextlib import ExitStack
    with tile.TileContext(nc) as tc, ExitStack() as stk:
        const = stk.enter_context(tc.tile_pool(name="const", bufs=1))
        wts = stk.enter_context(tc.tile_pool(name="wts", bufs=1))
        state = stk.enter_context(tc.tile_pool(name="state", bufs=1))
        sb = stk.enter_context(tc.tile_pool(name="sb", bufs=3))
        gsb = stk.enter_context(tc.tile_pool(name="gsb", bufs=2))
        ps = stk.enter_context(tc.tile_pool(name="ps", bufs=2, space="PSUM"))

        ident = const.tile([128, 128], F32)
        make_identity(nc, ident)
        if has_gate_bias:
            ones_row = const.tile([1, 128], F16)
            nc.vector.memset(ones_row, 1.0)

        det32 = state.tile([128, DET], F32)
        detT = state.tile([128, DET], F16)
        qmT = state.tile([128, STO], F16)
        hT = state.tile([128, EMB], F16)
        qhT = state.tile([128, EMB], F16)

        for l in (2, 1, 0):
            T = TS[l]
            w_ps = wts.tile([128, EMB], F16, tag="w_ps")
            nc.sync.dma_start(out=w_ps, in_=inp[f"wps{l}"][:, :])
            w_ih = wts.tile([128, KE * G], F16, tag="w_ih")
            nc.sync.dma_start(out=w_ih[:, :].rearrange("p (k g) -> p k g", k=KE), in_=inp[f"wih{l}"].rearrange("k p g -> p k g"))
            w_hh = wts.tile([128, KD * G], F16, tag="w_hh")
            nc.sync.dma_start(out=w_hh[:, :].rearrange("p (k g) -> p k g", k=KD), in_=inp[f"whh{l}"].rearrange("k p g -> p k g"))
            w_q = wts.tile([128, KQ * EMB], F16, tag="w_q")
            nc.sync.dma_start(out=w_q[:, :].rearrange("p (k e) -> p k e", k=KQ), in_=inp[f"wq{l}"].rearrange("k p e -> p k e"))
            w_qm = wts.tile([128, KE * STO], F16, tag="w_qm")
            nc.sync.dma_start(out=w_qm[:, :].rearrange("p (k s) -> p k s", k=KE), in_=inp[f"wqm{l}"].rearrange("k p s -> p k s"))
            b_p = wts.tile([128, KE], F32, tag="b_p")
            nc.sync.dma_start(out=b_p, in_=inp[f"bp{l}"].rearrange("(m p) -> p m", p=128))
            b_q = wts.tile([128, KE], F32, tag="b_q")
            nc.sync.dma_start(out=b_q, in_=inp[f"bq{l}"].rearrange("(m p) -> p m", p=128))
            b_qm = wts.tile([128, 1], F32, tag="b_qm")
            nc.sync.dma_start(out=b_qm, in_=inp[f"bqm{l}"].rearrange("(m p) -> p m", p=128))
            if has_gate_bias:
                b_g = wts.tile([1, 4096], F16, tag="b_g")
                nc.sync.dma_start(out=b_g, in_=inp[f"bg{l}"].rearrange("(o g) -> o g", o=1))

            # ---- precompute hctx[l] from parent detT store ----
            if l != 2:
                w_pc = wts.tile([128, KD * EMB], F16, tag="w_pc")
                nc.sync.dma_start(out=w_pc[:, :].rearrange("p (k e) -> p k e", k=KD), in_=inp[f"wpc{l}"].rearrange("k p e -> p k e"))
                Tp = TS[l + 1]
                src = detT_store[l + 1]
                dst = hctx[l]
                for sc in range(Tp // 4):
                    s4 = sc * 4
                    rhs = gsb.tile([128, KD * 512], F16, tag="pre_rhs")
                    for k in range(KD):
                        nc.sync.dma_start(
                            out=rhs[:, k * 512:(k + 1) * 512].rearrange("p (s b) -> p s b", s=4),
                            in_=src[s4:s4 + 4, k, :, :].rearrange("s p b -> p s b"),
                        )
                    pg = ps.tile([128, 2048], F32, tag="pg")
                    for m in range(KE):
                        for k in range(KD):
                            nc.tensor.matmul(
                                pg[:, m * 512:(m + 1) * 512],
                                w_pc[:, (k * KE + m) * 128:(k * KE + m) * 128 + 128],
                                rhs[:, k * 512:(k + 1) * 512],
                                start=(k == 0),
                                stop=(k == KD - 1),
                            )
                    hc = gsb.tile([128, 2048], F32, tag="pre_hc")
                    for m in range(KE):
                        nc.scalar.copy(out=hc[:, m * 512:(m + 1) * 512], in_=pg[:, m * 512:(m + 1) * 512])
                    for sl in range(4):
                        nc.scalar.dma_start(
                            out=dst[s4 + sl, :, :, :].rearrange("m p b -> p m b"),
                            in_=hc[:, :].rearrange("p (m s b) -> p m s b", m=KE, s=4)[:, :, sl, :],
                        )

            # ---- scan ----
            nc.vector.memset(det32, 0.0)
            nc.vector.memset(detT, 0.0)
            nc.vector.memset(qmT, 0.0)
            has_ctx = l != 2
            is_out = l == 0

            def step_body(t, eidx):
                obst = sb.tile([128, OBS], F16, tag="obst")
                nc.sync.dma_start(
                    out=obst[:, :].rearrange("p (k b) -> p k b", k=KE),
                    in_=inp[f"obst{l}"][ds(t, 1), :, :, :].rearrange("o k p b -> (o p) k b"),
                )
                # --- prior (feature-major) ---
                ppr = ps.tile([128, EMB], F32, tag="pg")
                for m in range(KE):
                    nc.tensor.matmul(
                        ppr[:, m * 128:(m + 1) * 128],
                        w_ps[:, m * 128:(m + 1) * 128],
                        qmT, start=True, stop=True,
                    )
                if has_ctx:
                    hcs = sb.tile([128, EMB], F32, tag="hcs")
                    nc.sync.dma_start(
                        out=hcs[:, :].rearrange("p (m b) -> p m b", m=KE),
                        in_=hctx[l][ds(eidx, 1), :, :, :].rearrange("o m p b -> (o p) m b"),
                    )
                    tpr = sb.tile([128, EMB], F32, tag="tpr")
                    nc.vector.tensor_add(out=tpr, in0=ppr, in1=hcs)
                    prsrc = tpr
                else:
                    prsrc = ppr
                for m in range(KE):
                    nc.scalar.activation(
                        out=hT[:, m * 128:(m + 1) * 128],
                        in_=prsrc[:, m * 128:(m + 1) * 128],
                        func=AF.Relu, bias=b_p[:, m:m + 1],
                    )
                # --- gi/gh in two det-column groups ---
                # pg_g layout (per group g, det cols g*512:(g+1)*512):
                #   [0:512]=r  [512:1024]=z  [1024:1536]=in  [1536:2048]=hn
                pgs = []
                for g in range(2):
                    pg_g = ps.tile([128, 2048], F32, tag="pg")
                    pgs.append(pg_g)
                    banks = [
                        (0, g * 512, True, True),             # r
                        (512, 1024 + g * 512, True, True),    # z
                        (1024, 2048 + g * 512, True, False),  # in (gi only)
                        (1536, 2048 + g * 512, False, True),  # hn (gh only)
                    ]
                    for bi, (dst, col, use_gi, use_gh) in enumerate(banks):
                        out_ap = pg_g[:, dst:dst + 512]
                        first = True
                        if use_gi:
                            for k in range(KE):
                                nc.tensor.matmul(
                                    out_ap, hT[:, k * 128:(k + 1) * 128],
                                    w_ih[:, k * G + col: k * G + col + 512],
                                    start=first,
                                    stop=(not use_gh and k == KE - 1 and not has_gate_bias),
                                )
                                first = False
                        if use_gh:
                            for k in range(KD):
                                nc.tensor.matmul(
                                    out_ap, detT[:, k * 128:(k + 1) * 128],
                                    w_hh[:, k * G + col: k * G + col + 512],
                                    start=first,
                                    stop=(k == KD - 1 and not has_gate_bias),
                                )
                                first = False
                        if has_gate_bias:
                            if col < 2048:
                                bcol = col
                            elif use_gi:
                                bcol = 2048 + g * 512
                            else:
                                bcol = 3072 + g * 512
                            nc.tensor.matmul(out_ap, ones_row, b_g[:, bcol:bcol + 512],
                                             start=False, stop=True)
                # --- gates: 4 blocks of 256 det cols, pipelined ---
                r_sb = gsb.tile([128, 1024], F32, tag="r_sb")
                z_sb = gsb.tile([128, 1024], F32, tag="z_sb")
                n_sb = gsb.tile([128, 1024], F32, tag="n_sb")
                d_sb = gsb.tile([128, 1024], F32, tag="d_sb")
                for g in range(2):
                    pg_g = pgs[g]
                    for bb in range(2):
                        bl = g * 2 + bb
                        c = bl * 256            # det col offset of this block
                        o = bb * 256            # offset within group tile
                        rs = r_sb[:, c:c + 256]
                        zs = z_sb[:, c:c + 256]
                        ns = n_sb[:, c:c + 256]
                        dsl = d_sb[:, c:c + 256]
                        nc.scalar.activation(out=rs, in_=pg_g[:, o:o + 256], func=AF.Sigmoid)
                        nc.scalar.activation(out=zs, in_=pg_g[:, 512 + o:512 + o + 256], func=AF.Sigmoid)
                        nc.vector.tensor_mul(out=ns, in0=rs, in1=pg_g[:, 1536 + o:1536 + o + 256])
                        nc.vector.tensor_add(out=ns, in0=ns, in1=pg_g[:, 1024 + o:1024 + o + 256])
                        nc.scalar.activation(out=ns, in_=ns, func=AF.Tanh)
                        nc.gpsimd.tensor_sub(out=dsl, in0=det32[:, c:c + 256], in1=ns)
                        nc.gpsimd.tensor_mul(out=dsl, in0=zs, in1=dsl)
                        nc.gpsimd.tensor_add(out=det32[:, c:c + 256], in0=ns, in1=dsl)
                    # transposes for this group's four 128-col tiles
                    ptp = ps.tile([128, 512], F32, tag="pg")
                    for kk in range(4):
                        k = g * 4 + kk
                        nc.tensor.transpose(ptp[:, kk * 128:(kk + 1) * 128],
                                            det32[:, k * 128:(k + 1) * 128], ident)
                    for kk in range(4):
                        k = g * 4 + kk
                        nc.vector.tensor_copy(out=detT[:, k * 128:(k + 1) * 128],
                                              in_=ptp[:, kk * 128:(kk + 1) * 128])
                if is_out:
                    nc.scalar.dma_start(
                        out=y[:, ds(t, 1), :].rearrange("p o d -> p (o d)"),
                        in_=det32,
                    )
                else:
                    nc.scalar.dma_start(
                        out=detT_store[l][ds(t, 1), :, :, :].rearrange("o k p b -> (o p) k b"),
                        in_=detT[:, :].rearrange("p (k b) -> p k b", k=KD),
                    )
                # --- posterior (feature-major); obs k-tiles first ---
                korder = list(range(KD, KQ)) + list(range(KD))
                ppo = ps.tile([128, EMB], F32, tag="pg")
                for m in range(KE):
                    for i, k in enumerate(korder):
                        rhs = detT[:, k * 128:(k + 1) * 128] if k < KD else obst[:, (k - KD) * 128:(k - KD + 1) * 128]
                        nc.tensor.matmul(
                            ppo[:, m * 128:(m + 1) * 128],
                            w_q[:, (k * KE + m) * 128:(k * KE + m) * 128 + 128],
                            rhs, start=(i == 0), stop=(i == KQ - 1),
                        )
                for m in range(KE):
                    nc.scalar.activation(
                        out=qhT[:, m * 128:(m + 1) * 128],
                        in_=ppo[:, m * 128:(m + 1) * 128],
                        func=AF.Relu, bias=b_q[:, m:m + 1],
                    )
                # --- head ---
                phd = ps.tile([128, STO], F32, tag="pg")
                for k in range(KE):
                    nc.tensor.matmul(
                        phd, w_qm[:, k * 128:(k + 1) * 128],
                        qhT[:, k * 128:(k + 1) * 128],
                        start=(k == 0), stop=(k == KE - 1),
                    )
                if has_bqm:
                    nc.scalar.activation(out=qmT, in_=phd, func=AF.Identity, bias=b_qm[:, 0:1])
                else:
                    nc.scalar.copy(out=qmT, in_=phd)

            hints = (mybir.EngineType.PE,)
            if not has_ctx:
                with tc.For_i(0, T // W, 1, hint_engines=hints) as sp:
                    for j in range(W):
                        step_body(sp * W + j, None)
            else:
                Tp = TS[l + 1]
                for a in range(T // Tp):
                    with tc.For_i(0, Tp // W, 1, hint_engines=hints) as sp:
                        for j in range(W):
                            e = sp * W + j
                            step_body(a * Tp + e, e)

    nc.compile()
    return nc


def prep_inputs(inputs, has_gate_bias=False):
    f16 = np.float16
    m = {}
    obs = [inputs["obs_l0"], inputs["obs_l1"], inputs["obs_l2"]]
    for l in range(3):
        T = TS[l]
        Wp, Wih, Whh = inputs["Wp"][l], inputs["Wih"][l], inputs["Whh"][l]
        Wq, Wqm = inputs["Wq"][l], inputs["Wqm"][l]
        m[f"wps{l}"] = np.ascontiguousarray(Wp[0:128]).astype(f16)
        if l != 2:
            m[f"wpc{l}"] = np.ascontiguousarray(Wp[128:1152].reshape(KD, 128, EMB)).astype(f16)
        m[f"wih{l}"] = np.ascontiguousarray(Wih.reshape(KE, 128, G)).astype(f16)
        m[f"whh{l}"] = np.ascontiguousarray(Whh.reshape(KD, 128, G)).astype(f16)
        m[f"wq{l}"] = np.ascontiguousarray(Wq.reshape(KQ, 128, EMB)).astype(f16)
        m[f"wqm{l}"] = np.ascontiguousarray(Wqm.reshape(KE, 128, STO)).astype(f16)
        o = np.asarray(obs[l], np.float32)
        m[f"obst{l}"] = np.ascontiguousarray(o.transpose(1, 2, 0).reshape(T, KE, 128, B)).astype(f16)
        m[f"bp{l}"] = np.ascontiguousarray(inputs["bp"][l], dtype=np.float32)
        m[f"bq{l}"] = np.ascontiguousarray(inputs["bq"][l], dtype=np.float32)
        m[f"bqm{l}"] = np.ascontiguousarray(inputs["bqm"][l], dtype=np.float32)
        if has_gate_bias:
            bih, bhh = np.asarray(inputs["bih"][l], np.float32), np.asarray(inputs["bhh"][l], np.float32)
            bg = np.concatenate([(bih + bhh)[0:2048], bih[2048:3072], bhh[2048:3072]])
            m[f"bg{l}"] = bg.astype(f16)
    return m


_CACHE = {}


def kernel(**inputs):
    inputs = {k: np.asarray(v) for k, v in inputs.items()}
    key = flags_for(inputs)
    if key not in _CACHE:
        _CACHE[key] = build_kernel(*key)
    nc = _CACHE[key]
    in_map = prep_inputs(inputs, key[0])
    res = run_bass_kernel_spmd(nc, [in_map for _ in range(8)], core_ids=list(range(8)))
    return res.results[0]["y"].astype(np.float32)


def flags_for(inputs):
    import numpy as _np
    return (
        bool(_np.any(inputs["bih"]) or _np.any(inputs["bhh"])),
        bool(_np.any(inputs["bqm"])),
        bool(_np.any(inputs["bp"]) or _np.any(inputs["bq"])),
    )



# revision 6
# speedup vs baseline: 1.0807x; 1.0643x over previous
"""CWVAE Bass kernel for trn2: 3-level RSSM scan, replicated across 8 cores.

Reference semantics (per level, T steps, ctx_t = parent_det[t mod Tp]):
  h     = relu(concat(sample, ctx_t) @ Wp + bp)
  det'  = GRUCell(h, det)            (torch gate order r,z,n)
  qh    = relu(concat(det', obs_t) @ Wq + bq)
  sample= qh @ Wqm + bqm
  out_t = det'

v2 design notes:
  - fp16 matmul inputs (same PE rate as bf16, 3 more mantissa bits)
  - gi/gh emitted in two 512-det-column groups; each group's PSUM tile packs
    [r | z | in | hn] for those det columns so gate math starts after the
    first group's 36 matmuls instead of after all 72
  - gate chain runs in 256-col blocks: ACT sigmoid/tanh, DVE mul/add,
    GpSimd blend, then per-group PE transposes -> detT (fp16)
  - posterior accumulates obs k-tiles first (k=8..11), then det k-tiles, so
    PE has ready work while late det blocks finish
  - per-step PSUM tiles rotate through one 2-slot pool tag (8 banks total)
"""
import sys

sys.path.insert(0, "/opt/trn_rl_repo")

import numpy as np

import concourse.bass as bass
import concourse.tile as tile
from concourse import bacc, mybir
from concourse.bass import ds
from concourse.bass_utils import run_bass_kernel_spmd
from concourse.masks import make_identity

F32 = mybir.dt.float32
F16 = mybir.dt.float16
AF = mybir.ActivationFunctionType

B = 128
DET = 1024
EMB = 512
STO = 128
OBS = 512
G = 3072
TS = [256, 64, 16]
KD = DET // 128   # 8
KE = EMB // 128   # 4
KQ = (DET + OBS) // 128  # 12
W = 16            # unroll (steps per inner loop body)


def build_kernel(has_gate_bias=False, has_bqm=False, has_pq_bias=True):
    nc = bacc.Bacc()

    inp = {}
    for l in range(3):
        T = TS[l]
        inp[f"wps{l}"] = nc.dram_tensor(f"wps{l}", [128, EMB], F16, kind="ExternalInput")
        if l != 2:
            inp[f"wpc{l}"] = nc.dram_tensor(f"wpc{l}", [KD, 128, EMB], F16, kind="ExternalInput")
        inp[f"wih{l}"] = nc.dram_tensor(f"wih{l}", [KE, 128, G], F16, kind="ExternalInput")
        inp[f"whh{l}"] = nc.dram_tensor(f"whh{l}", [KD, 128, G], F16, kind="ExternalInput")
        inp[f"wq{l}"] = nc.dram_tensor(f"wq{l}", [KQ, 128, EMB], F16, kind="ExternalInput")
        inp[f"wqm{l}"] = nc.dram_tensor(f"wqm{l}", [KE, 128, STO], F16, kind="ExternalInput")
        inp[f"obst{l}"] = nc.dram_tensor(f"obst{l}", [T, KE, 128, B], F16, kind="ExternalInput")
        inp[f"bp{l}"] = nc.dram_tensor(f"bp{l}", [EMB], F32, kind="ExternalInput")
        inp[f"bq{l}"] = nc.dram_tensor(f"bq{l}", [EMB], F32, kind="ExternalInput")
        inp[f"bqm{l}"] = nc.dram_tensor(f"bqm{l}", [STO], F32, kind="ExternalInput")
        if has_gate_bias:
            # packed [brz(2048) | bin(1024) | bhn(1024)] in fp16
            inp[f"bg{l}"] = nc.dram_tensor(f"bg{l}", [4096], F16, kind="ExternalInput")

    y = nc.dram_tensor("y", [B, TS[0], DET], F32, kind="ExternalOutput")

    detT_store = {
        2: nc.dram_tensor("detT2", [TS[2], KD, 128, B], F16),
        1: nc.dram_tensor("detT1", [TS[1], KD, 128, B], F16),
    }
    hctx = {
        1: nc.dram_tensor("hctx1", [TS[2], KE, 128, B], F32),
        0: nc.dram_tensor("hctx0", [TS[1], KE, 128, B], F32),
    }

    from contextlib import ExitStack
    with tile.TileContext(nc) as tc, ExitStack() as stk:
        const = stk.enter_context(tc.tile_pool(name="const", bufs=1))
        wts = stk.enter_context(tc.tile_pool(name="wts", bufs=1))
        state = stk.enter_context(tc.tile_pool(name="state", bufs=1))
        sb = stk.enter_context(tc.tile_pool(name="sb", bufs=3))
        gsb = stk.enter_context(tc.tile_pool(name="gsb", bufs=2))
        ps = stk.enter_context(tc.tile_pool(name="ps", bufs=2, space="PSUM"))

        ident = const.tile([128, 128], F32)
        make_identity(nc, ident)
        if has_gate_bias:
            ones_row = const.tile([1, 128], F16)
            nc.vector.memset(ones_row, 1.0)

        det32 = state.tile([128, DET], F32)
        detT = state.tile([128, DET], F16)
        qmT = state.tile([128, STO], F16)
        hT = state.tile([128, EMB], F16)
        qhT = state.tile([128, EMB], F16)

        for l in (2, 1, 0):
            T = TS[l]
            w_ps = wts.tile([128, EMB], F16, tag="w_ps")
            nc.sync.dma_start(out=w_ps, in_=inp[f"wps{l}"][:, :])
            w_ih = wts.tile([128, KE * G], F16, tag="w_ih")
            nc.sync.dma_start(out=w_ih[:, :].rearrange("p (k g) -> p k g", k=KE), in_=inp[f"wih{l}"].rearrange("k p g -> p k g"))
            w_hh = wts.tile([128, KD * G], F16, tag="w_hh")
            nc.sync.dma_start(out=w_hh[:, :].rearrange("p (k g) -> p k g", k=KD), in_=inp[f"whh{l}"].rearrange("k p g -> p k g"))
            w_q = wts.tile([128, KQ * EMB], F16, tag="w_q")
            nc.sync.dma_start(out=w_q[:, :].rearrange("p (k e) -> p k e", k=KQ), in_=inp[f"wq{l}"].rearrange("k p e -> p k e"))
            w_qm = wts.tile([128, KE * STO], F16, tag="w_qm")
            nc.sync.dma_start(out=w_qm[:, :].rearrange("p (k s) -> p k s", k=KE), in_=inp[f"wqm{l}"].rearrange("k p s -> p k s"))
            b_p = wts.tile([128, KE], F32, tag="b_p")
            nc.sync.dma_start(out=b_p, in_=inp[f"bp{l}"].rearrange("(m p) -> p m", p=128))
            b_q = wts.tile([128, KE], F32, tag="b_q")
            nc.sync.dma_start(out=b_q, in_=inp[f"bq{l}"].rearrange("(m p) -> p m", p=128))
            b_qm = wts.tile([128, 1], F32, tag="b_qm")
            nc.sync.dma_start(out=b_qm, in_=inp[f"bqm{l}"].rearrange("(m p) -> p m", p=128))
            if has_gate_bias:
                b_g = wts.tile([1, 4096], F16, tag="b_g")
                nc.sync.dma_start(out=b_g, in_=inp[f"bg{l}"].rearrange("(o g) -> o g", o=1))

            # ---- precompute hctx[l] from parent detT store ----
            if l != 2:
                w_pc = wts.tile([128, KD * EMB], F16, tag="w_pc")
                nc.sync.dma_start(out=w_pc[:, :].rearrange("p (k e) -> p k e", k=KD), in_=inp[f"wpc{l}"].rearrange("k p e -> p k e"))
                Tp = TS[l + 1]
                src = detT_store[l + 1]
                dst = hctx[l]
                for sc in range(Tp // 4):
                    s4 = sc * 4
                    rhs = gsb.tile([128, KD * 512], F16, tag="pre_rhs")
                    for k in range(KD):
                        nc.sync.dma_start(
                            out=rhs[:, k * 512:(k + 1) * 512].rearrange("p (s b) -> p s b", s=4),
                            in_=src[s4:s4 + 4, k, :, :].rearrange("s p b -> p s b"),
                        )
                    pg = ps.tile([128, 2048], F32, tag="pg")
                    for m in range(KE):
                        for k in range(KD):
                            nc.tensor.matmul(
                                pg[:, m * 512:(m + 1) * 512],
                                w_pc[:, (k * KE + m) * 128:(k * KE + m) * 128 + 128],
                                rhs[:, k * 512:(k + 1) * 512],
                                start=(k == 0),
                                stop=(k == KD - 1),
                            )
                    hc = gsb.tile([128, 2048], F32, tag="pre_hc")
                    for m in range(KE):
                        nc.scalar.copy(out=hc[:, m * 512:(m + 1) * 512], in_=pg[:, m * 512:(m + 1) * 512])
                    for sl in range(4):
                        nc.scalar.dma_start(
                            out=dst[s4 + sl, :, :, :].rearrange("m p b -> p m b"),
                            in_=hc[:, :].rearrange("p (m s b) -> p m s b", m=KE, s=4)[:, :, sl, :],
                        )

            # ---- scan ----
            nc.vector.memset(det32, 0.0)
            nc.vector.memset(detT, 0.0)
            nc.vector.memset(qmT, 0.0)
            has_ctx = l != 2
            is_out = l == 0

            def step_body(t, eidx):
                obst = sb.tile([128, OBS], F16, tag="obst")
                nc.sync.dma_start(
                    out=obst[:, :].rearrange("p (k b) -> p k b", k=KE),
                    in_=inp[f"obst{l}"][ds(t, 1), :, :, :].rearrange("o k p b -> (o p) k b"),
                )
                # --- prior (feature-major) ---
                ppr = ps.tile([128, EMB], F32, tag="pg")
                for m in range(KE):
                    nc.tensor.matmul(
                        ppr[:, m * 128:(m + 1) * 128],
                        w_ps[:, m * 128:(m + 1) * 128],
                        qmT, start=True, stop=True,
                    )
                if has_ctx:
                    hcs = sb.tile([128, EMB], F32, tag="hcs")
                    nc.sync.dma_start(
                        out=hcs[:, :].rearrange("p (m b) -> p m b", m=KE),
                        in_=hctx[l][ds(eidx, 1), :, :, :].rearrange("o m p b -> (o p) m b"),
                    )
                    tpr = sb.tile([128, EMB], F32, tag="tpr")
                    nc.vector.tensor_add(out=tpr, in0=ppr, in1=hcs)
                    prsrc = tpr
                else:
                    prsrc = ppr
                for m in range(KE):
                    if m < 2:
                        nc.scalar.activation(
                            out=hT[:, m * 128:(m + 1) * 128],
                            in_=prsrc[:, m * 128:(m + 1) * 128],
                            func=AF.Relu, bias=b_p[:, m:m + 1],
                        )
                    else:
                        nc.vector.tensor_scalar(
                            out=hT[:, m * 128:(m + 1) * 128],
                            in0=prsrc[:, m * 128:(m + 1) * 128],
                            scalar1=b_p[:, m:m + 1], scalar2=0.0,
                            op0=mybir.AluOpType.add, op1=mybir.AluOpType.max,
                        )
                # --- gi/gh in two det-column groups ---
                # pg_g layout (per group g, det cols g*512:(g+1)*512):
                #   [0:512]=r  [512:1024]=z  [1024:1536]=in  [1536:2048]=hn
                pgs = []
                for g in range(2):
                    pg_g = ps.tile([128, 2048], F32, tag="pg")
                    pgs.append(pg_g)
                    banks = [
                        (0, g * 512, True, True),             # r
                        (512, 1024 + g * 512, True, True),    # z
                        (1024, 2048 + g * 512, True, False),  # in (gi only)
                        (1536, 2048 + g * 512, False, True),  # hn (gh only)
                    ]
                    for bi, (dst, col, use_gi, use_gh) in enumerate(banks):
                        out_ap = pg_g[:, dst:dst + 512]
                        first = True
                        if use_gh:
                            # gh first: needs only last step's detT, so PE
                            # streams it while ACT finishes this step's hT relu
                            for k in range(KD):
                                nc.tensor.matmul(
                                    out_ap, detT[:, k * 128:(k + 1) * 128],
                                    w_hh[:, k * G + col: k * G + col + 512],
                                    start=first,
                                    stop=(not use_gi and k == KD - 1 and not has_gate_bias),
                                )
                                first = False
                        if use_gi:
                            for k in range(KE):
                                nc.tensor.matmul(
                                    out_ap, hT[:, k * 128:(k + 1) * 128],
                                    w_ih[:, k * G + col: k * G + col + 512],
                                    start=first,
                                    stop=(k == KE - 1 and not has_gate_bias),
                                )
                                first = False
                        if has_gate_bias:
                            if col < 2048:
                                bcol = col
                            elif use_gi:
                                bcol = 2048 + g * 512
                            else:
                                bcol = 3072 + g * 512
                            nc.tensor.matmul(out_ap, ones_row, b_g[:, bcol:bcol + 512],
                                             start=False, stop=True)
                # --- gates: 4 blocks of 256 det cols, pipelined ---
                r_sb = gsb.tile([128, 1024], F32, tag="r_sb")
                z_sb = gsb.tile([128, 1024], F32, tag="z_sb")
                n_sb = gsb.tile([128, 1024], F32, tag="n_sb")
                d_sb = gsb.tile([128, 1024], F32, tag="d_sb")
                for g in range(2):
                    pg_g = pgs[g]
                    for bb in range(2):
                        bl = g * 2 + bb
                        c = bl * 256            # det col offset of this block
                        o = bb * 256            # offset within group tile
                        rs = r_sb[:, c:c + 256]
                        zs = z_sb[:, c:c + 256]
                        ns = n_sb[:, c:c + 256]
                        dsl = d_sb[:, c:c + 256]
                        nc.scalar.activation(out=rs, in_=pg_g[:, o:o + 256], func=AF.Sigmoid)
                        nc.scalar.activation(out=zs, in_=pg_g[:, 512 + o:512 + o + 256], func=AF.Sigmoid)
                        nc.vector.tensor_mul(out=ns, in0=rs, in1=pg_g[:, 1536 + o:1536 + o + 256])
                        nc.vector.tensor_add(out=ns, in0=ns, in1=pg_g[:, 1024 + o:1024 + o + 256])
                        nc.scalar.activation(out=ns, in_=ns, func=AF.Tanh)
                        nc.gpsimd.tensor_sub(out=dsl, in0=det32[:, c:c + 256], in1=ns)
                        nc.gpsimd.tensor_mul(out=dsl, in0=zs, in1=dsl)
                        nc.gpsimd.tensor_add(out=det32[:, c:c + 256], in0=ns, in1=dsl)
                    # transposes for this group's four 128-col tiles
                    ptp = ps.tile([128, 512], F32, tag="pg")
                    for kk in range(4):
                        k = g * 4 + kk
                        nc.tensor.transpose(ptp[:, kk * 128:(kk + 1) * 128],
                                            det32[:, k * 128:(k + 1) * 128], ident)
                    for kk in range(4):
                        k = g * 4 + kk
                        nc.vector.tensor_copy(out=detT[:, k * 128:(k + 1) * 128],
                                              in_=ptp[:, kk * 128:(kk + 1) * 128])
                if is_out:
                    nc.scalar.dma_start(
                        out=y[:, ds(t, 1), :].rearrange("p o d -> p (o d)"),
                        in_=det32,
                    )
                else:
                    nc.scalar.dma_start(
                        out=detT_store[l][ds(t, 1), :, :, :].rearrange("o k p b -> (o p) k b"),
                        in_=detT[:, :].rearrange("p (k b) -> p k b", k=KD),
                    )
                # --- posterior (feature-major); obs k-tiles first ---
                korder = list(range(KD, KQ)) + list(range(KD))
                ppo = ps.tile([128, EMB], F32, tag="pg")
                for m in range(KE):
                    for i, k in enumerate(korder):
                        rhs = detT[:, k * 128:(k + 1) * 128] if k < KD else obst[:, (k - KD) * 128:(k - KD + 1) * 128]
                        nc.tensor.matmul(
                            ppo[:, m * 128:(m + 1) * 128],
                            w_q[:, (k * KE + m) * 128:(k * KE + m) * 128 + 128],
                            rhs, start=(i == 0), stop=(i == KQ - 1),
                        )
                for m in range(KE):
                    if m < 2:
                        nc.scalar.activation(
                            out=qhT[:, m * 128:(m + 1) * 128],
                            in_=ppo[:, m * 128:(m + 1) * 128],
                            func=AF.Relu, bias=b_q[:, m:m + 1],
                        )
                    else:
                        nc.vector.tensor_scalar(
                            out=qhT[:, m * 128:(m + 1) * 128],
                            in0=ppo[:, m * 128:(m + 1) * 128],
                            scalar1=b_q[:, m:m + 1], scalar2=0.0,
                            op0=mybir.AluOpType.add, op1=mybir.AluOpType.max,
                        )
                # --- head ---
                phd = ps.tile([128, STO], F32, tag="pg")
                for k in range(KE):
                    nc.tensor.matmul(
                        phd, w_qm[:, k * 128:(k + 1) * 128],
                        qhT[:, k * 128:(k + 1) * 128],
                        start=(k == 0), stop=(k == KE - 1),
                    )
                if has_bqm:
                    nc.scalar.activation(out=qmT, in_=phd, func=AF.Identity, bias=b_qm[:, 0:1])
                else:
                    nc.scalar.copy(out=qmT, in_=phd)

            hints = (mybir.EngineType.PE,)
            if not has_ctx:
                with tc.For_i(0, T // W, 1, hint_engines=hints) as sp:
                    for j in range(W):
                        step_body(sp * W + j, None)
            else:
                Tp = TS[l + 1]
                for a in range(T // Tp):
                    with tc.For_i(0, Tp // W, 1, hint_engines=hints) as sp:
                        for j in range(W):
                            e = sp * W + j
                            step_body(a * Tp + e, e)

    nc.compile()
    return nc


def prep_inputs(inputs, has_gate_bias=False):
    f16 = np.float16
    m = {}
    obs = [inputs["obs_l0"], inputs["obs_l1"], inputs["obs_l2"]]
    for l in range(3):
        T = TS[l]
        Wp, Wih, Whh = inputs["Wp"][l], inputs["Wih"][l], inputs["Whh"][l]
        Wq, Wqm = inputs["Wq"][l], inputs["Wqm"][l]
        m[f"wps{l}"] = np.ascontiguousarray(Wp[0:128]).astype(f16)
        if l != 2:
            m[f"wpc{l}"] = np.ascontiguousarray(Wp[128:1152].reshape(KD, 128, EMB)).astype(f16)
        m[f"wih{l}"] = np.ascontiguousarray(Wih.reshape(KE, 128, G)).astype(f16)
        m[f"whh{l}"] = np.ascontiguousarray(Whh.reshape(KD, 128, G)).astype(f16)
        m[f"wq{l}"] = np.ascontiguousarray(Wq.reshape(KQ, 128, EMB)).astype(f16)
        m[f"wqm{l}"] = np.ascontiguousarray(Wqm.reshape(KE, 128, STO)).astype(f16)
        o = np.asarray(obs[l], np.float32)
        m[f"obst{l}"] = np.ascontiguousarray(o.transpose(1, 2, 0).reshape(T, KE, 128, B)).astype(f16)
        m[f"bp{l}"] = np.ascontiguousarray(inputs["bp"][l], dtype=np.float32)
        m[f"bq{l}"] = np.ascontiguousarray(inputs["bq"][l], dtype=np.float32)
        m[f"bqm{l}"] = np.ascontiguousarray(inputs["bqm"][l], dtype=np.float32)
        if has_gate_bias:
            bih, bhh = np.asarray(inputs["bih"][l], np.float32), np.asarray(inputs["bhh"][l], np.float32)
            bg = np.concatenate([(bih + bhh)[0:2048], bih[2048:3072], bhh[2048:3072]])
            m[f"bg{l}"] = bg.astype(f16)
    return m


_CACHE = {}


def kernel(**inputs):
    inputs = {k: np.asarray(v) for k, v in inputs.items()}
    key = flags_for(inputs)
    if key not in _CACHE:
        _CACHE[key] = build_kernel(*key)
    nc = _CACHE[key]
    in_map = prep_inputs(inputs, key[0])
    res = run_bass_kernel_spmd(nc, [in_map for _ in range(8)], core_ids=list(range(8)))
    return res.results[0]["y"].astype(np.float32)


def flags_for(inputs):
    import numpy as _np
    return (
        bool(_np.any(inputs["bih"]) or _np.any(inputs["bhh"])),
        bool(_np.any(inputs["bqm"])),
        bool(_np.any(inputs["bp"]) or _np.any(inputs["bq"])),
    )

